# revision 23
# baseline (speedup 1.0000x reference)
"""CNLinkPredictor Trainium2 kernel.

Edge-sharded across 8 NeuronCores (1024 target edges each); x, adj, and the
MLP weights are replicated. Per core:
  A) h = x + MLP(x) in transposed layout: host supplies xT, stage A is
     matmul-only on PE (bf16, 512-node moving dim), fused bias+ReLU on the
     scalar engine, residual on DVE into wide [128, 2048] hT tiles, then one
     xbar DMA-transpose per (c-half, 4-group) writes h back to natural
     layout (column order (hh, kt, c2): h[node=kt*128+p, ch=hh*128+c2]).
  B) the adjacency is BIT-PACKED on the host into u16 words with k-stride
     256: adjp[n, s*256+w] bit j = adj[n, s*4096 + j*256 + w].  Per edge
     block (128 edges): one indirect-DMA gather per endpoint fetches the
     full 1024-byte packed row pair; a DVE bitwise-AND gives the packed
     common-neighbor mask; ONE 2-byte xbar transpose per block moves it
     k-major (64 KB instead of the 1 MB an unpacked transpose would be).
     Per (bit j, half s): ONE DVE bitvec op (and 1<<j, shl to bit 14)
     unpacks all 8 blocks at once into the bf16 bit pattern of 2.0 (the
     host pre-halves xcn_w1 to compensate; strided out AP groups columns
     into contiguous kt tiles), then - via bitcast - 8 matmuls accumulate
     xcnT[c, e] += h[kt]^T cn[kt] with h tiles stationary.  xcnT lands
     already transposed for stage C - no further transposes needed.
  C) xi*xj gathered per block (bf16 rows), DVE product, one packed xbar
     transpose per block; edge MLPs in transposed layout (512-edge
     groups).  The xij layer (independent of xcn) is emitted INTO the
     stage-B j-loop so PE/Act digest it during B; only the xcn-dependent
     chain (u1 -> u2 -> z -> v -> out) trails the last B matmul.

Scheduling rules this kernel follows (learned from TimelineSim traces):
  - Engine SEQs are in-order FIFOs and a waiting instruction parks on the
    queue head: all dependency-free DMAs (xT loads) are emitted before any
    dependent transpose on the same queue.
  - PSUM is 8 banks: stage A uses 4 (scoped), then stage B accumulators 4
    + stage C layer tiles 3 + output 1 coexist.

Hardware pitfalls this kernel works around:
  - This walrus build accepts at most ONE sync-wait per instruction
    (_apply_tile_patch splits the Tile tail drain; _split_multi_waits
    hoists extra waits onto same-engine NoOps).
  - Concurrent 4-byte DMA traffic corrupts in-flight 2-byte xbar
    DMA-transposes, so every steady-state transfer is <= 2 bytes/element
    (u16 packed adjacency, bf16 everything else); the few f32/int32 loads
    happen up front and the single f32 store happens after the last
    transpose.
  - The bitwise DVE path cannot cast dtypes (and rejects float dtypes),
    so the unpack writes u16 tiles holding bf16 bit patterns (0x4000=2.0)
    and the matmul reads them through AP.bitcast(bf16).
  - Indirect DMA consumes ONE index per out partition (the partition's
    whole span streams contiguously from the indexed row), so gathers are
    per-128-edge-block.
"""

import numpy as np
import ml_dtypes

N = 8192
C = 256
E = 8192
NCORES = 8
EL = E // NCORES          # edges per core
P = 128
NB = EL // P              # edge blocks per core (8)
NS = 2                    # k halves (4096 each)
NW = 256                  # u16 words per half per row
NJ = 16                   # bits per word
AGRP = 512                # stage-A node group
NGW = 4                   # stage-A transpose super-groups (4 groups each)
CGRP = 4                  # stage-C blocks per group (512 edges)

_CACHE = {}
TRACE = False
LAST_RESULT = None


def _apply_tile_patch():
    """Split the Tile tail-drain's multi-sem wait onto individual SP nops."""
    from concourse.tile import TileContext
    from concourse.vector_clock import ScopedClock

    if getattr(TileContext, "_drain_patched", False):
        return

    def _patched(self, tick_clock, wait_clock):
        nc = self.nc
        collector = nc.sync.nop()
        wait_clock.add_sem_waits(
            collector.ins, ScopedClock({None: tick_clock.global_clock})
        )
        si = collector.ins.sync_info
        waits = list(si.on_wait) if si is not None and si.on_wait else []
        if si is not None and len(waits) > 1:
            name_to_handle = {h.name: h for h in self.sems.allocated().values()}
            si.on_wait = [waits[0]]
            for w in waits[1:]:
                op = {
                    "sem-ge-imm": "sem-ge",
                    "sem-eq-imm": "sem-eq",
                    "sem-le-imm": "sem-le",
                }.get(str(w.wait_mode), "sem-ge")
                nc.sync.nop().wait_op(name_to_handle[w.ant_name], w.wait_value, op)
        nc.sync.drain()
        nc.all_engine_barrier()
        assert self.sems is not None
        popped = nc._tile_sem_poison_stack.pop()
        assert popped is self._sem_poison
        nc.clear_and_free_semaphores(list(self.sems.allocated().values()))
        nc.all_engine_barrier()

    TileContext._drain_and_barrier = _patched
    TileContext._drain_patched = True


def _split_multi_waits(nc):
    """Hoist extra sync-waits onto same-engine NoOps (sequential waits ==
    ANDed waits); this walrus build allows one wait per instruction."""
    import concourse.mybir as mybir

    cnt = 0
    for fn in nc.m.functions:
        for bb in fn.blocks:
            out = []
            for inst in bb.instructions:
                si = getattr(inst, "sync_info", None)
                waits = list(si.on_wait) if si is not None and si.on_wait else []
                if len(waits) > 1:
                    for w in waits[:-1]:
                        nop = mybir.InstNoOp(name=f"ws-{cnt}", ins=[], outs=[])
                        cnt += 1
                        nop.engine = inst.engine
                        nop.sync_info = mybir.SyncInfo(on_wait=[w], on_update=[])
                        out.append(nop)
                    si.on_wait = [waits[-1]]
                out.append(inst)
            bb.instructions = out
    return nc


def _build(split_waits=True):
    import concourse.bass as bass
    import concourse.mybir as mybir
    from concourse.tile import TileContext

    _apply_tile_patch()

    f32 = mybir.dt.float32
    bf16 = mybir.dt.bfloat16
    u16 = mybir.dt.uint16
    i32 = mybir.dt.int32
    Relu = mybir.ActivationFunctionType.Relu
    Ident = mybir.ActivationFunctionType.Identity
    MUL = mybir.AluOpType.mult
    ADD = mybir.AluOpType.add
    ANDB = mybir.AluOpType.bitwise_and
    SHR = mybir.AluOpType.logical_shift_right
    SHL = mybir.AluOpType.logical_shift_left
    MAX = mybir.AluOpType.max

    nc = bass.Bass(num_swdge_queues=4)

    xT_d = nc.dram_tensor("xT", [C, N], bf16, kind="ExternalInput")
    x_d = nc.dram_tensor("x", [N, C], bf16, kind="ExternalInput")
    adjp_d = nc.dram_tensor("adjp", [N, NS * NW], u16, kind="ExternalInput")
    idx_d = nc.dram_tensor("idx", [2, EL], i32, kind="ExternalInput")
    # all bf16 matmul weights in one load: 6 [C,C] ws + lin_w2 padded to 2 cols
    WN = ["xlin_w1", "xlin_w2", "xcn_w1", "xcn_w2", "xij_w", "lin_w1"]
    wall_d = nc.dram_tensor("wall", [P, 12 * C + 2], bf16, kind="ExternalInput")
    # f32 consts: 6 bias pairs + beta + lin_b2
    bnames = ["xlin_b1", "xlin_b2", "xcn_b1", "xcn_b2", "xij_b", "lin_b1"]
    ball_d = nc.dram_tensor("ball", [P, 2 * len(bnames) + 2], f32,
                            kind="ExternalInput")
    out_d = nc.dram_tensor("out", [1, EL], f32, kind="ExternalOutput")

    _swq = [0]

    def _rr(inst):
        q = _swq[0] % 4
        _swq[0] += 1
        if q:
            inst.ins.queue = f"qPoolDynamic{q}"
        return inst

    with TileContext(nc) as tc:
        with (
            tc.tile_pool(name="const", bufs=1) as pK,
            tc.tile_pool(name="hpool", bufs=1) as pH,
            tc.tile_pool(name="adj", bufs=1) as pAdj,
            tc.tile_pool(name="xtw", bufs=1) as pXT,
            tc.tile_pool(name="unp", bufs=3) as pU,
            tc.tile_pool(name="edge", bufs=1) as pE,
            tc.tile_pool(name="stC", bufs=2) as pC,
        ):
            # ---- constants ----
            idx_sb = pK.tile([P, 2 * NB], i32, tag="idx_sb", name="idx_sb")
            nc.sync.dma_start(
                out=idx_sb[:].rearrange("p (t b) -> p t b", t=2),
                in_=idx_d[:, :].rearrange("t (b p) -> p t b", p=P),
            )
            ii = [idx_sb[:, b:b + 1] for b in range(NB)]
            jj = [idx_sb[:, NB + b:NB + b + 1] for b in range(NB)]

            wall = pK.tile([P, 12 * C + 2], bf16, tag="wall", name="wall")
            nc.sync.dma_start(out=wall[:, :4 * C], in_=wall_d[:, :4 * C])
            w_sb = {}
            for q, n in enumerate(WN):
                w_sb[n] = [wall[:, (2 * q) * C:(2 * q + 1) * C],
                           wall[:, (2 * q + 1) * C:(2 * q + 2) * C]]
            lw2_sb = [wall[:, 12 * C:12 * C + 1], wall[:, 12 * C + 1:12 * C + 2]]

            xTw = {}

            def load_xtw(gw):
                for h in range(2):
                    t = pXT.tile([P, 4 * AGRP], bf16, tag=f"xTw{gw}_{h}",
                                 name=f"xTw{gw}_{h}")
                    nc.sync.dma_start(
                        out=t[:],
                        in_=xT_d[h * P:(h + 1) * P,
                                 gw * 4 * AGRP:(gw + 1) * 4 * AGRP],
                    )
                    xTw[(gw, h)] = t

            load_xtw(0)
            nc.sync.dma_start(out=wall[:, 4 * C:], in_=wall_d[:, 4 * C:])
            ball = pK.tile([P, 2 * len(bnames) + 2], f32, tag="ball", name="ball")
            nc.sync.dma_start(out=ball[:], in_=ball_d[:, :])
            b_sb = {n: ball[:, 2 * q:2 * q + 2] for q, n in enumerate(bnames)}
            beta_sb = ball[:, 12:13]
            lb2_sb = ball[:, 13:14]

            out_row = pK.tile([1, EL], f32, tag="out_row", name="out_row")
            # natural-layout h: column = hh*N + kt*128 + c2 holds
            # h[node = kt*128 + p, channel = hh*128 + c2]
            h_all = pH.tile([P, 2 * N], bf16, tag="h_all", name="h_all")
            h_view = h_all[:].rearrange("p (hh kt c) -> p hh kt c", hh=2, c=P)

            load_xtw(1)
            load_xtw(2)
            load_xtw(3)

            # ---- stage B gathers + packed transposes (AND after transpose:
            # the transposes then wait only on their own gather, keeping the
            # DMA queues flowing) ----
            # T*[p, b, 2s+wc, e] = word (s, wc*128+p) of edge (b,e)'s row
            Ti_all = pH.tile([P, NB * 4 * P], u16, tag="Ti_all", name="Ti_all")
            Ti_view = Ti_all[:].rearrange("p (b ch e) -> p b ch e", b=NB, e=P)
            Tj_all = pH.tile([P, NB * 4 * P], u16, tag="Tj_all", name="Tj_all")
            Tj_view = Tj_all[:].rearrange("p (b ch e) -> p b ch e", b=NB, e=P)
            T_view = Ti_view
            exi = [pE.tile([P, C], bf16, tag=f"xi{b}", name=f"xi{b}")
                   for b in range(NB)]
            exj = [pE.tile([P, C], bf16, tag=f"xj{b}", name=f"xj{b}")
                   for b in range(NB)]
            eprod = [pE.tile([P, C], bf16, tag=f"prod{b}", name=f"prod{b}")
                     for b in range(NB)]
            ai_t, aj_t = {}, {}
            for b in range(NB):
                ai = pAdj.tile([P, NS * NW], u16, tag=f"ai{b}", name=f"ai{b}")
                _rr(nc.gpsimd.indirect_dma_start(
                    out=ai[:], out_offset=None, in_=adjp_d[:, :],
                    in_offset=bass.IndirectOffsetOnAxis(ap=ii[b][:, :1], axis=0),
                ))
                aj = pAdj.tile([P, NS * NW], u16, tag=f"aj{b}", name=f"aj{b}")
                _rr(nc.gpsimd.indirect_dma_start(
                    out=aj[:], out_offset=None, in_=adjp_d[:, :],
                    in_offset=bass.IndirectOffsetOnAxis(ap=jj[b][:, :1], axis=0),
                ))
                ai_t[b], aj_t[b] = ai, aj

            def emit_adj_tr(b):
                # SP-queue transposes, dispatched after group b's emission so
                # their gather waits are already (almost) satisfied.
                nc.sync.dma_start_transpose(
                    out=Ti_view[:, b, :, :], in_=ai_t[b][:])
                nc.sync.dma_start_transpose(
                    out=Tj_view[:, b, :, :], in_=aj_t[b][:])

            # ---- stage C gathers (Pool queue, after adjacency gathers) ----
            for b in range(NB):
                _rr(nc.gpsimd.indirect_dma_start(
                    out=exi[b][:], out_offset=None, in_=x_d[:, :],
                    in_offset=bass.IndirectOffsetOnAxis(ap=ii[b][:, :1], axis=0),
                ))
                _rr(nc.gpsimd.indirect_dma_start(
                    out=exj[b][:], out_offset=None, in_=x_d[:, :],
                    in_offset=bass.IndirectOffsetOnAxis(ap=jj[b][:, :1], axis=0),
                ))

            # ---- stage B unpack+matmul chunk emitter ----
            ps_x = {}

            def emit_b(s, j):
                uj = pU.tile([P, 2 * EL], u16, tag="uj", name=f"uj{j}_{s}")
                # single bitvec op: isolate bit j and move it to bit 14 = the
                # bf16 bit pattern of 2.0 (host pre-halves xcn_w1 to
                # compensate).  out col = wc*1024+b*128+e; in iterates
                # (b, wc, e) = T_all's (b, ch=2s+wc, e)
                ts_kw = dict(
                    out=uj[:].rearrange("p (wc b e) -> p b wc e", wc=2, e=P),
                    in0=T_view[:, :, 2 * s:2 * s + 2, :],
                    scalar1=1 << j,
                )
                if j < 14:
                    nc.vector.tensor_scalar(
                        scalar2=14 - j, op0=ANDB, op1=SHL, **ts_kw)
                elif j == 14:
                    nc.vector.tensor_scalar(
                        scalar2=None, op0=ANDB, **ts_kw)
                else:
                    nc.vector.tensor_scalar(
                        scalar2=1, op0=ANDB, op1=SHR, **ts_kw)
                cj = uj[:].bitcast(bf16)
                for wc in range(2):
                    kt = s * 32 + 2 * j + wc
                    for hh in range(2):
                        for eh in range(2):
                            nc.tensor.matmul(
                                ps_x[(hh, eh)][:],
                                h_view[:, hh, kt, :],
                                cj[:, wc * EL + eh * 512:
                                   wc * EL + (eh + 1) * 512],
                                start=(j == 0 and s == 0 and wc == 0),
                                stop=(j == NJ - 1 and s == 1 and wc == 1),
                            )

            B_AFTER = {g: [(0, 4 * (g - 12) + k) for k in range(4)]
                       for g in range(12, 16)}

            # ---- stage A ----
            # psA uses 4 PSUM banks; the sibling psB/psC/psO pools (opened
            # below, tiles allocated later) take the other 4 so stage-B
            # matmuls can interleave with stage A on PE.
            from contextlib import ExitStack
            _stk = ExitStack()
            _stkB = ExitStack()
            with tc.tile_pool(name="stA", bufs=4) as pA, \
                 tc.tile_pool(name="hT", bufs=2) as pHT:
                psA = _stk.enter_context(
                    tc.tile_pool(name="psA1", bufs=8, space="PSUM"))
                hTw = {}
                for g in range(4 * NGW):
                    if g == 12:
                        # groups 0-11 drained: shrink stage-A psum to 4 banks
                        # and hand 4 to the stage-B accumulators; emit the
                        # (now unparked) packed-mask ANDs
                        for b_ in range(NB):
                            nc.vector.tensor_tensor(
                                out=T_view[:, b_, :, :],
                                in0=Ti_view[:, b_, :, :],
                                in1=Tj_view[:, b_, :, :], op=ANDB)
                        _stk.close()
                        psB = _stkB.enter_context(
                            tc.tile_pool(name="psB", bufs=1, space="PSUM"))
                        psA = _stk.enter_context(
                            tc.tile_pool(name="psA2", bufs=4, space="PSUM"))
                        for hh in range(2):
                            for eh in range(2):
                                ps_x[(hh, eh)] = psB.tile(
                                    [P, EL // 2], f32, tag=f"psx{hh}{eh}",
                                    name=f"psx{hh}{eh}")
                    gw, gl = g // 4, g % 4
                    if gl == 0:
                        for h in range(2):
                            hTw[h] = pHT.tile([P, 4 * AGRP], bf16,
                                              tag=f"hTw{h}", name=f"hTw{gw}_{h}")
                    xT = [xTw[(gw, h)][:, gl * AGRP:(gl + 1) * AGRP]
                          for h in range(2)]
                    y1T = []
                    for h in range(2):
                        ps = psA.tile([P, AGRP], f32, tag="psmm",
                                      name=f"psA1_{g}{h}")
                        nc.tensor.matmul(
                            ps[:], w_sb["xlin_w1"][0][:, h * P:(h + 1) * P],
                            xT[0], start=True, stop=False,
                        )
                        nc.tensor.matmul(
                            ps[:], w_sb["xlin_w1"][1][:, h * P:(h + 1) * P],
                            xT[1], start=False, stop=True,
                        )
                        t = pA.tile([P, AGRP], bf16, tag=f"y1T{h}",
                                    name=f"y1T{h}_{g}")
                        nc.scalar.activation(
                            t[:], ps[:], Relu, bias=b_sb["xlin_b1"][:, h:h + 1]
                        )
                        y1T.append(t)
                    for h in range(2):
                        ps = psA.tile([P, AGRP], f32, tag="psmm",
                                      name=f"psA2_{g}{h}")
                        nc.tensor.matmul(
                            ps[:], w_sb["xlin_w2"][0][:, h * P:(h + 1) * P],
                            y1T[0][:], start=True, stop=False,
                        )
                        nc.tensor.matmul(
                            ps[:], w_sb["xlin_w2"][1][:, h * P:(h + 1) * P],
                            y1T[1][:], start=False, stop=True,
                        )
                        y2 = pA.tile([P, AGRP], bf16, tag="y2T", name=f"y2T{h}_{g}")
                        nc.scalar.activation(
                            y2[:], ps[:], Relu, bias=b_sb["xlin_b2"][:, h:h + 1]
                        )
                        nc.vector.tensor_tensor(
                            out=hTw[h][:, gl * AGRP:(gl + 1) * AGRP],
                            in0=xT[h], in1=y2[:], op=ADD,
                        )
                    if gl == 3:
                        for h in range(2):
                            nc.sync.dma_start_transpose(
                                out=h_view[:, h, gw * 16:(gw + 1) * 16, :],
                                in_=hTw[h][:],
                            )
                    if g < NB:
                        emit_adj_tr(g)
                    for sb_, jb_ in B_AFTER.get(g, []):
                        emit_b(sb_, jb_)

            # ---- edge products + transposes (DVE/SP, ready mid j-loop) ----
            prodT_all = pH.tile([P, NB * 2 * P], bf16, tag="prodT",
                                name="prodT_all")
            prodT_v = prodT_all[:].rearrange("p (b ch e) -> p b ch e",
                                             b=NB, e=P)
            for b in range(NB):
                nc.vector.tensor_tensor(
                    out=eprod[b][:], in0=exi[b][:], in1=exj[b][:], op=MUL
                )
                nc.sync.dma_start_transpose(
                    out=prodT_v[:, b, :, :], in_=eprod[b][:])

            # ---- remaining stage B chunks + interleaved stage-C xij ----
            _stk.close()  # psA2 banks freed for psC/psO
            with tc.tile_pool(name="psC", bufs=3, space="PSUM") as psC, \
                 tc.tile_pool(name="psO", bufs=1, space="PSUM") as psO:

                def mlp_layer(grp, rhs_pair, wname, bname, outtag):
                    W = 512
                    outs = []
                    for h in range(2):
                        ps = psC.tile([P, W], f32, tag="psc",
                                      name=f"psc_{grp}_{outtag}{h}")
                        nc.tensor.matmul(
                            ps[:], w_sb[wname][0][:, h * P:(h + 1) * P],
                            rhs_pair[0], start=True, stop=False,
                        )
                        nc.tensor.matmul(
                            ps[:], w_sb[wname][1][:, h * P:(h + 1) * P],
                            rhs_pair[1], start=False, stop=True,
                        )
                        t = pC.tile([P, W], bf16, tag=f"{outtag}{h}",
                                    name=f"{outtag}{h}_{grp}")
                        nc.scalar.activation(
                            t[:], ps[:], Relu, bias=b_sb[bname][:, h:h + 1]
                        )
                        outs.append(t)
                    return outs

                xijT = {}
                for jr in range(NJ):
                    emit_b(1, jr)
                for jr in (11,):
                        for grp in range(EL // 512):
                            xijT[grp] = mlp_layer(
                                grp,
                                [prodT_v[:, grp * CGRP:(grp + 1) * CGRP, ch, :]
                                 for ch in range(2)],
                                "xij_w", "xij_b", "xijT")

                xcn_sb = pH.tile([P, 2 * EL], bf16, tag="xcn", name="xcn_sb")
                for hh in range(2):
                    for eh in range(2):
                        nc.vector.tensor_copy(
                            xcn_sb[:, hh * EL + eh * 512:
                                   hh * EL + (eh + 1) * 512],
                            ps_x[(hh, eh)][:])

                # ---- stage C xcn-dependent chain ----
                for grp in range(EL // 512):
                    W = 512
                    xcn_rhs = [
                        xcn_sb[:, ch * EL + grp * W:ch * EL + (grp + 1) * W]
                        for ch in range(2)
                    ]
                    u1T = mlp_layer(grp, xcn_rhs, "xcn_w1", "xcn_b1", "u1T")
                    u2T = mlp_layer(grp, [u1T[0][:], u1T[1][:]],
                                    "xcn_w2", "xcn_b2", "u2T")
                    zT = []
                    for h in range(2):
                        zb = pC.tile([P, W], bf16, tag=f"zb{h}",
                                     name=f"zb{h}_{grp}")
                        nc.vector.tensor_tensor(
                            out=zb[:], in0=u2T[h][:],
                            in1=beta_sb.to_broadcast([P, W]), op=MUL,
                        )
                        zt = pC.tile([P, W], bf16, tag=f"zT{h}",
                                     name=f"zT{h}_{grp}")
                        nc.vector.tensor_tensor(
                            out=zt[:], in0=zb[:], in1=xijT[grp][h][:], op=ADD
                        )
                        zT.append(zt)
                    vT = mlp_layer(grp, [zT[0][:], zT[1][:]],
                                   "lin_w1", "lin_b1", "vT")
                    pso = psO.tile([1, W], f32, tag="pso", name=f"pso{grp}")
                    nc.tensor.matmul(
                        pso[:], lw2_sb[0][:], vT[0][:], start=True, stop=False
                    )
                    nc.tensor.matmul(
                        pso[:], lw2_sb[1][:], vT[1][:], start=False, stop=True
                    )
                    nc.scalar.activation(
                        out_row[0:1, grp * W:(grp + 1) * W], pso[:],
                        Ident, bias=lb2_sb[0:1, 0:1],
                    )

            _stkB.close()
            nc.sync.dma_start(out=out_d[:, :], in_=out_row[0:1, :])

    return _split_multi_waits(nc) if split_waits else nc


def kernel(**inputs):
    from concourse.bass_utils import run_bass_kernel_spmd

    if "nc" not in _CACHE:
        _CACHE["nc"] = _build()
    nc = _CACHE["nc"]

    x = np.ascontiguousarray(inputs["x"], dtype=np.float32)
    adj = np.asarray(inputs["adj"])
    # pack adjacency bits into u16 words, k-stride 256 within each 4096-half:
    # adjp[n, s*256+w] bit j = adj[n, s*4096 + j*256 + w]
    A = (adj != 0).astype(np.uint16).reshape(N, 2, 16, 256)
    adjp = np.zeros((N, 2, 256), np.uint16)
    for j in range(16):
        adjp |= A[:, :, j, :] << j
    adjp = np.ascontiguousarray(adjp.reshape(N, 512))
    tar = np.asarray(inputs["tar_ei"]).astype(np.int32)

    def btile(b):
        return np.ascontiguousarray(
            np.asarray(b, dtype=np.float32).reshape(2, P).T)

    WN = ["xlin_w1", "xlin_w2", "xcn_w1", "xcn_w2", "xij_w", "lin_w1"]
    wall = np.zeros((P, 12 * C + 2), np.float32)
    for q, n in enumerate(WN):
        w = np.asarray(inputs[n], dtype=np.float32)  # [C, C]
        if n == "xcn_w1":
            w = w * 0.5  # stage-B cn values are 2.0, not 1.0
        wall[:, (2 * q) * C:(2 * q + 1) * C] = w[:P, :]
        wall[:, (2 * q + 1) * C:(2 * q + 2) * C] = w[P:, :]
    lw2 = np.asarray(inputs["lin_w2"], dtype=np.float32).reshape(C, 1)
    wall[:, 12 * C] = lw2[:P, 0]
    wall[:, 12 * C + 1] = lw2[P:, 0]

    bnames = ["xlin_b1", "xlin_b2", "xcn_b1", "xcn_b2", "xij_b", "lin_b1"]
    ball = np.zeros((P, 2 * len(bnames) + 2), np.float32)
    for q, n in enumerate(bnames):
        ball[:, 2 * q:2 * q + 2] = btile(inputs[n])
    ball[:, 12] = np.asarray(inputs["beta"]).reshape(-1)[0]
    ball[:, 13] = np.asarray(inputs["lin_b2"]).reshape(-1)[0]

    common = {
        "x": x.astype(ml_dtypes.bfloat16),
        "xT": np.ascontiguousarray(x.T).astype(ml_dtypes.bfloat16),
        "adjp": adjp,
        "wall": np.ascontiguousarray(wall).astype(ml_dtypes.bfloat16),
        "ball": np.ascontiguousarray(ball),
    }

    in_maps = []
    for c in range(NCORES):
        m = dict(common)
        m["idx"] = np.ascontiguousarray(tar[:, c * EL:(c + 1) * EL])
        in_maps.append(m)

    res = run_bass_kernel_spmd(
        nc, in_maps, core_ids=list(range(NCORES)), trace=TRACE
    )
    global LAST_RESULT
    LAST_RESULT = res
    out = np.concatenate(
        [res.results[c]["out"].reshape(EL, 1) for c in range(NCORES)], axis=0
    )
    return out.astype(np.float32)
